# revision 1
# baseline (speedup 1.0000x reference)
"""DRGCN message-passing kernel for 8 Trainium2 NeuronCores.

Strategy: shard by destination-node range (12500 nodes/core) so each core
computes its output rows fully locally (no collectives). Host preprocesses
edges into a padded, (tile, relation)-sorted slot layout and pre-gathers
source features into a streaming-friendly layout; the device does the
segment mean (one-hot scatter matmuls with an in-pass count column), the
basis-decomposed per-relation weight composition, per-relation transforms,
and the root/bias term.
"""
import numpy as np

N_NODES = 100000
IN_C = 64
OUT_C = 64
NUM_REL = 8
R2 = 2 * NUM_REL            # 16
NUM_M, NUM_N, NUM_O = 4, 2, 1
NUM_BASES = NUM_M + NUM_N * NUM_REL + NUM_O * R2  # 36
P = 128
NCORES = 8
NPC = N_NODES // NCORES     # 12500 nodes per core
NTILES = (NPC + P - 1) // P  # 98
NPAD = NTILES * P           # 12544
J = 32                      # groups per z-chunk DMA
SENTINEL = 999.0


def _build_weight_mask():
    m = np.zeros((R2, NUM_BASES), dtype=np.float32)
    m[:, :NUM_M] = 1.0
    for row_i in range(R2):
        for col_i in range(NUM_REL):
            if row_i == col_i or row_i == col_i + NUM_REL:
                c = col_i * NUM_N
                m[row_i, NUM_M + c:NUM_M + c + NUM_N] = 1.0
        for col_i in range(R2):
            if row_i == col_i:
                s = NUM_M + NUM_N * NUM_REL + col_i * NUM_O
                m[row_i, s:s + NUM_O] = 1.0
    return m


def _host_prep(x, edge_index, edge_type):
    """Returns per-core arrays + shared group structure."""
    src = np.concatenate([edge_index[0], edge_index[1]]).astype(np.int64)
    dst = np.concatenate([edge_index[1], edge_index[0]]).astype(np.int64)
    rel = np.concatenate([edge_type, edge_type + NUM_REL]).astype(np.int64)

    core = dst // NPC
    dst_local = dst - core * NPC
    tile_id = dst_local // P
    key = tile_id * R2 + rel  # run id within core, tile-major rel-inner

    per_core = []
    run_counts = np.zeros((NCORES, NTILES * R2), dtype=np.int64)
    for c in range(NCORES):
        m = core == c
        s_c, dl_c, k_c = src[m], dst_local[m], key[m]
        order = np.argsort(k_c, kind="stable")
        s_c, dl_c, k_c = s_c[order], dl_c[order], k_c[order]
        run_counts[c] = np.bincount(k_c, minlength=NTILES * R2)
        per_core.append((s_c, dl_c, k_c))

    g_run = (np.max(run_counts, axis=0) + P - 1) // P  # groups per run, shared
    G = int(g_run.sum())
    run_g0 = np.concatenate([[0], np.cumsum(g_run)])[:-1]

    x_aug = np.concatenate(
        [x.astype(np.float32), np.ones((N_NODES, 1), np.float32)], axis=1)  # [N,65]

    cores_data = []
    for c in range(NCORES):
        s_c, dl_c, k_c = per_core[c]
        cnt_c = run_counts[c]
        run_starts = np.concatenate([[0], np.cumsum(cnt_c)])[:-1]
        slot_src = np.zeros(G * P, dtype=np.int64)
        slot_dst = np.full(G * P, SENTINEL, dtype=np.float32)
        # place each run's edges at its slot offset
        nz = np.nonzero(cnt_c)[0]
        for kr in nz:
            n = cnt_c[kr]
            s0 = run_g0[kr] * P
            e0 = run_starts[kr]
            slot_src[s0:s0 + n] = s_c[e0:e0 + n]
            t = kr // R2
            slot_dst[s0:s0 + n] = (dl_c[e0:e0 + n] - t * P).astype(np.float32)
        # z stream layout [128, G*65]: row p holds concat over g of x_aug[src[g*128+p]]
        z = x_aug[slot_src].reshape(G, P, IN_C + 1).transpose(1, 0, 2)
        z = np.ascontiguousarray(z).reshape(P, G * (IN_C + 1))
        dst_cols = slot_dst.reshape(G, P).T.copy()  # [128, G]
        # padded transposed x slice for the root term
        xt = np.zeros((IN_C, NPAD), dtype=np.float32)
        xt[:, :NPC] = x[c * NPC:(c + 1) * NPC].T
        cores_data.append({"z": z, "dstc": dst_cols, "xt": xt})
    return cores_data, g_run, G


def _build_program(g_run, G, fp16=False, repeat=1, nopipe=True, v2=True):
    import concourse.tile as tile
    from concourse import bass, bacc, mybir
    from contextlib import ExitStack

    f32 = mybir.dt.float32
    agg_dt = mybir.dt.float16 if fp16 else f32
    nc = bacc.Bacc("TRN2", target_bir_lowering=False, debug=False,
                   num_devices=NCORES)
    NCHUNK = (G + J - 1) // J
    GP = NCHUNK * J  # padded group count for chunked streaming

    z_dram = nc.declare_dram_parameter("z", [P, GP * (IN_C + 1)], agg_dt, isOutput=False)
    dst_dram = nc.declare_dram_parameter("dstc", [P, GP], f32, isOutput=False)
    xt_dram = nc.declare_dram_parameter("xt", [IN_C, NPAD], f32, isOutput=False)
    wgt_dram = nc.declare_dram_parameter("wgt", [NUM_BASES, IN_C * OUT_C], f32, isOutput=False)
    mct_dram = nc.declare_dram_parameter("mct", [NUM_BASES, R2], f32, isOutput=False)
    root_dram = nc.declare_dram_parameter("root", [IN_C, OUT_C], f32, isOutput=False)
    bias_dram = nc.declare_dram_parameter("bias", [OUT_C, 1], f32, isOutput=False)
    iota_dram = nc.declare_dram_parameter("iota", [P, P], agg_dt, isOutput=False)
    ident_dram = nc.declare_dram_parameter("ident", [P, P], agg_dt, isOutput=False)
    out_dram = nc.declare_dram_parameter("out", [OUT_C, NPAD], f32, isOutput=True)

    w_scratch = nc.dram_tensor("w_scratch", [R2, IN_C, OUT_C], f32)

    with tile.TileContext(nc) as tc:
        with ExitStack() as ctx:
            const_p = ctx.enter_context(tc.tile_pool(name="const", bufs=1, space="SBUF"))
            zchunk_p = ctx.enter_context(tc.tile_pool(name="zchunk", bufs=4, space="SBUF"))
            oh_p = ctx.enter_context(tc.tile_pool(name="oh", bufs=8, space="SBUF"))
            small_p = ctx.enter_context(tc.tile_pool(name="small", bufs=8, space="SBUF"))
            aggt_p = ctx.enter_context(tc.tile_pool(name="aggt", bufs=4, space="SBUF"))
            out_p = ctx.enter_context(tc.tile_pool(name="outs", bufs=3, space="SBUF"))
            ps_agg_p = ctx.enter_context(tc.tile_pool(name="psagg", bufs=2, space="PSUM"))
            ps_t_p = ctx.enter_context(tc.tile_pool(name="pst", bufs=2, space="PSUM"))
            ps_out_p = ctx.enter_context(tc.tile_pool(name="psout", bufs=2, space="PSUM"))

            iota_t = const_p.tile([P, P], agg_dt)
            nc.sync.dma_start(out=iota_t[:], in_=iota_dram[:])
            ident_t = const_p.tile([P, P], agg_dt)
            nc.sync.dma_start(out=ident_t[:], in_=ident_dram[:])
            root_t = const_p.tile([IN_C, OUT_C], f32)
            nc.sync.dma_start(out=root_t[:], in_=root_dram[:])
            bias_t = const_p.tile([OUT_C, 1], f32)
            nc.sync.dma_start(out=bias_t[:], in_=bias_dram[:])

            # ---- weight composition: W[r] = (mask*comp @ weight_flat)[r] ----
            mct_t = const_p.tile([NUM_BASES, R2], f32)
            nc.sync.dma_start(out=mct_t[:], in_=mct_dram[:])
            wgt_t = const_p.tile([NUM_BASES, IN_C * OUT_C], f32)
            nc.sync.dma_start(out=wgt_t[:], in_=wgt_dram[:])
            w_all = const_p.tile([R2, IN_C * OUT_C], f32)
            for k in range(IN_C * OUT_C // 512):
                ps_w = ps_agg_p.tile([R2, 512], f32, space="PSUM")
                nc.tensor.matmul(out=ps_w[:], lhsT=mct_t[:],
                                 rhs=wgt_t[:, k * 512:(k + 1) * 512],
                                 start=True, stop=True)
                nc.vector.tensor_copy(out=w_all[:, k * 512:(k + 1) * 512], in_=ps_w[:])
            nc.sync.dma_start(out=w_scratch[:, :, :], in_=w_all[:])
            w_tiles = []
            for r in range(R2):
                w_r32 = const_p.tile([IN_C, OUT_C], f32, name=f"w_r32_{r}")
                nc.sync.dma_start(out=w_r32[:], in_=w_scratch[r, :, :])
                if fp16:
                    w_r = const_p.tile([IN_C, OUT_C], agg_dt, name=f"w_r{r}")
                    nc.vector.tensor_copy(out=w_r[:], in_=w_r32[:])
                else:
                    w_r = w_r32
                w_tiles.append(w_r)

            # ---- main loop ----
            CW = IN_C + 1  # 65 columns per group in the z stream
            zt = None
            dt = None
            g_cum = np.concatenate([[0], np.cumsum(g_run)])
            pending = [None]

            def make_tail(ps_agg, r, ps_out, stop_flag, fin_t):
                def tail(ps_agg=ps_agg, r=r, ps_out=ps_out,
                         stop_flag=stop_flag, fin_t=fin_t):
                    cnt_cl = small_p.tile([P, 1], f32, name="cnt_cl")
                    nc.vector.tensor_scalar(out=cnt_cl[:], in0=ps_agg[:, IN_C:CW],
                                            scalar1=1.0, scalar2=None,
                                            op0=mybir.AluOpType.max)
                    recip = small_p.tile([P, 1], f32, name="recip")
                    nc.vector.reciprocal(out=recip[:], in_=cnt_cl[:])
                    agg = aggt_p.tile([P, IN_C], agg_dt, name="agg")
                    if v2:
                        nc.scalar.activation(
                            out=agg[:], in_=ps_agg[:, 0:IN_C],
                            func=mybir.ActivationFunctionType.Identity,
                            scale=recip[:, 0:1])
                    else:
                        nc.vector.tensor_scalar(out=agg[:], in0=ps_agg[:, 0:IN_C],
                                                scalar1=recip[:, 0:1], scalar2=None,
                                                op0=mybir.AluOpType.mult)
                    ps_t = ps_t_p.tile([IN_C, P], agg_dt, space="PSUM", name="ps_t")
                    nc.tensor.transpose(out=ps_t[:], in_=agg[:], identity=ident_t[:])
                    aggT = aggt_p.tile([IN_C, P], agg_dt, name="aggT")
                    nc.scalar.activation(out=aggT[:], in_=ps_t[:],
                                         func=mybir.ActivationFunctionType.Copy)
                    nc.tensor.matmul(out=ps_out[:], lhsT=w_tiles[r], rhs=aggT[:],
                                     start=False, stop=stop_flag)
                    if fin_t is not None:
                        o_sb = out_p.tile([OUT_C, P], f32, name="o_sb")
                        nc.scalar.activation(
                            out=o_sb[:], in_=ps_out[:],
                            func=mybir.ActivationFunctionType.Identity,
                            bias=bias_t[:, 0:1])
                        nc.sync.dma_start(
                            out=out_dram[:, fin_t * P:(fin_t + 1) * P], in_=o_sb[:])
                return tail

            for rep in range(repeat):
              zt_ch = -1
              for t in range(NTILES):
                  xt_t = small_p.tile([IN_C, P], f32, name="xt_t")
                  nc.sync.dma_start(out=xt_t[:], in_=xt_dram[:, t * P:(t + 1) * P])
                  ps_out = ps_out_p.tile([OUT_C, P], f32, space="PSUM", name="ps_out")
                  runs = [r for r in range(R2) if g_run[t * R2 + r] > 0]
                  nc.tensor.matmul(out=ps_out[:], lhsT=root_t[:], rhs=xt_t[:],
                                   start=True, stop=(len(runs) == 0))
                  if not runs:
                      o_sb = out_p.tile([OUT_C, P], f32, name="o_sb")
                      nc.scalar.activation(
                          out=o_sb[:], in_=ps_out[:],
                          func=mybir.ActivationFunctionType.Identity,
                          bias=bias_t[:, 0:1])
                      nc.sync.dma_start(out=out_dram[:, t * P:(t + 1) * P],
                                        in_=o_sb[:])
                      continue
                  for ri, r in enumerate(runs):
                      kr = t * R2 + r
                      g0 = int(g_cum[kr])
                      ng = int(g_run[kr])
                      ps_agg = ps_agg_p.tile([P, CW], f32, space="PSUM", name="ps_agg")
                      for k in range(ng):
                          g = g0 + k
                          ch, gl = g // J, g % J
                          if ch != zt_ch:
                              zt = zchunk_p.tile([P, J * CW], agg_dt, name="zt")
                              nc.sync.dma_start(
                                  out=zt[:], in_=z_dram[:, ch * J * CW:(ch + 1) * J * CW])
                              dt = zchunk_p.tile([P, J], f32, name="dt")
                              nc.sync.dma_start(
                                  out=dt[:], in_=dst_dram[:, ch * J:(ch + 1) * J])
                              zt_ch = ch
                          oh = oh_p.tile([P, P], agg_dt, name="oh")
                          oh_eng = nc.gpsimd if (v2 and k % 4 == 3) else nc.vector
                          oh_eng.tensor_scalar(
                              out=oh[:], in0=iota_t[:], scalar1=dt[:, gl:gl + 1],
                              scalar2=None, op0=mybir.AluOpType.is_equal)
                          nc.tensor.matmul(out=ps_agg[:], lhsT=oh[:],
                                           rhs=zt[:, gl * CW:(gl + 1) * CW],
                                           start=(k == 0), stop=(k == ng - 1))
                      if pending[0] is not None:
                          pending[0]()
                      pending[0] = make_tail(ps_agg, r, ps_out,
                                             stop_flag=(ri == len(runs) - 1),
                                             fin_t=(t if ri == len(runs) - 1 else None))
                      if nopipe:
                          pending[0]()
                          pending[0] = None
            if pending[0] is not None:
                pending[0]()
                pending[0] = None

    nc.compile()
    return nc


def prepare(x, edge_index, edge_type, weight, comp, root, bias,
            fp16=None, repeat=1, nopipe=True, v2=True):
    import os
    if fp16 is None:
        fp16 = not bool(os.environ.get("DRGCN_F32"))
    x = np.asarray(x, dtype=np.float32)
    edge_index = np.asarray(edge_index)
    edge_type = np.asarray(edge_type)
    weight = np.asarray(weight, dtype=np.float32)
    comp = np.asarray(comp, dtype=np.float32)
    root = np.asarray(root, dtype=np.float32)
    bias = np.asarray(bias, dtype=np.float32)

    cores_data, g_run, G = _host_prep(x, edge_index, edge_type)
    nc = _build_program(g_run, G, fp16=fp16, repeat=repeat, nopipe=nopipe, v2=v2)

    mask = _build_weight_mask()
    mct = np.ascontiguousarray((mask * comp).T)          # [36, 16]
    wgt = weight.reshape(NUM_BASES, IN_C * OUT_C)
    zdt = np.float16 if fp16 else np.float32
    iota = np.tile(np.arange(P, dtype=zdt)[None, :], (P, 1))
    ident = np.eye(P, dtype=zdt)
    bias_col = bias.reshape(OUT_C, 1)

    NCHUNK = (G + J - 1) // J
    GP = NCHUNK * J
    in_maps = []
    for c in range(NCORES):
        d = cores_data[c]
        z = d["z"].astype(zdt)
        if GP != G:  # pad stream to chunk multiple
            zp = np.zeros((P, GP * (IN_C + 1)), zdt)
            zp[:, :G * (IN_C + 1)] = z
            z = zp
            dc = np.full((P, GP), SENTINEL, np.float32)
            dc[:, :G] = d["dstc"]
        else:
            dc = d["dstc"]
        in_maps.append({
            "z": z, "dstc": dc, "xt": d["xt"], "wgt": wgt, "mct": mct,
            "root": root, "bias": bias_col, "iota": iota, "ident": ident,
        })

    return nc, in_maps


def assemble(results):
    out = np.empty((N_NODES, OUT_C), dtype=np.float32)
    for c in range(NCORES):
        out[c * NPC:(c + 1) * NPC] = results[c]["out"][:, :NPC].T
    return out


def kernel(x, edge_index, edge_type, weight, comp, root, bias):
    from concourse.bass_utils import run_bass_kernel_spmd

    nc, in_maps = prepare(x, edge_index, edge_type, weight, comp, root, bias)
    res = run_bass_kernel_spmd(nc, in_maps, core_ids=list(range(NCORES)))
    return assemble(res.results)



# revision 14
# speedup vs baseline: 1.3997x; 1.3997x over previous
"""DRGCN message-passing kernel for 8 Trainium2 NeuronCores.

Strategy: shard by destination-node range (12500 nodes/core) so each core
computes its output rows fully locally (no collectives). Host preprocesses
edges into a padded, (tile, relation)-sorted slot layout, pre-gathers source
features (pre-scaled by the segment 1/count) into a streaming z layout.

Device inner loop per dst tile (128 nodes):
  - for each relation r (16), accumulate agg_r^T = sum_slots z_slot one-hot
    scatter matmuls in [in=64, dst=128] orientation: matmul(lhsT=z[slot,64],
    rhs=onehot[slot,dst]) -> PSUM. Two relations stack into one [128,128]
    PSUM region (partition halves), four pairs per PSUM bank ("quad").
  - one activation copy per quad PSUM->SBUF (fp16), then one matmul per
    relation-pair: lhsT=[W_r0;W_r1] [128,64], rhs=agg pair [128,128],
    accumulating into ps_out[64,128] on top of the root term.
  - one-hot matrices are built 4 groups at a time with a single DVE
    tensor_tensor(is_equal) against a tiled iota, using a stride-65 +
    broadcast access pattern on the dst-code column of the z stream.
"""
import numpy as np

N_NODES = 100000
IN_C = 64
OUT_C = 64
NUM_REL = 8
R2 = 2 * NUM_REL            # 16
NUM_M, NUM_N, NUM_O = 4, 2, 1
NUM_BASES = NUM_M + NUM_N * NUM_REL + NUM_O * R2  # 36
P = 128
NCORES = 8
NPC = N_NODES // NCORES     # 12500 nodes per core
NTILES = (NPC + P - 1) // P  # 98
NPAD = NTILES * P            # 12544
NRUNS = NTILES * R2          # 1568 runs per core
J = 32                       # groups per z-chunk DMA (multiple of OB)
OB = 8                       # one-hot build batch (groups per DVE instr)
CW = IN_C + 3                # 67 cols/group: 64 vals + dst f16 + dst f32(2)


def _build_weight_mask():
    m = np.zeros((R2, NUM_BASES), dtype=np.float32)
    m[:, :NUM_M] = 1.0
    for row_i in range(R2):
        for col_i in range(NUM_REL):
            if row_i == col_i or row_i == col_i + NUM_REL:
                c = col_i * NUM_N
                m[row_i, NUM_M + c:NUM_M + c + NUM_N] = 1.0
        for col_i in range(R2):
            if row_i == col_i:
                s = NUM_M + NUM_N * NUM_REL + col_i * NUM_O
                m[row_i, s:s + NUM_O] = 1.0
    return m


def _host_prep(x, edge_index, edge_type):
    """Sort/pad edges per core, pre-gather scaled source features.

    Returns per-core dicts {z, xt} plus the shared group structure g_run, G.
    """
    src = np.concatenate([edge_index[0], edge_index[1]]).astype(np.int64)
    dst = np.concatenate([edge_index[1], edge_index[0]]).astype(np.int64)
    rel = np.concatenate([edge_type, edge_type + NUM_REL]).astype(np.int64)

    core = dst // NPC
    dst_local = dst - core * NPC
    key = (dst_local // P) * R2 + rel          # run id within core

    run_counts = np.zeros((NCORES, NRUNS), dtype=np.int64)
    per_core = []
    for c in range(NCORES):
        m = core == c
        s_c, dl_c, k_c = src[m], dst_local[m], key[m]
        order = np.argsort(k_c, kind="stable")
        s_c, dl_c, k_c = s_c[order], dl_c[order], k_c[order]
        run_counts[c] = np.bincount(k_c, minlength=NRUNS)
        per_core.append((s_c, dl_c, k_c))

    g_run = (np.max(run_counts, axis=0) + P - 1) // P    # shared across cores
    G = int(g_run.sum())
    GP = ((G + J - 1) // J) * J
    run_g0 = np.concatenate([[0], np.cumsum(g_run)])[:-1]

    xf = x.astype(np.float32)
    cores_data = []
    for c in range(NCORES):
        s_c, dl_c, k_c = per_core[c]
        cnt_c = run_counts[c]
        run_starts = np.concatenate([[0], np.cumsum(cnt_c)])[:-1]
        rank = np.arange(len(k_c)) - run_starts[k_c]
        slot = run_g0[k_c] * P + rank                    # global slot id
        dst_in_tile = dl_c - (k_c // R2) * P             # 0..127
        # per-(rel,dst) counts -> fold 1/cnt into the gathered features
        subkey = k_c * P + dst_in_tile
        cnt_edge = np.bincount(subkey, minlength=NRUNS * P)[subkey]
        vals = xf[s_c] * (1.0 / cnt_edge)[:, None].astype(np.float32)

        # chunk layout: [J groups x 64 value cols | J dst f16 | J dst f32]
        zv = np.zeros((GP * P, IN_C), dtype=np.float16)
        zv[slot, :] = vals.astype(np.float16)
        zd = np.zeros((GP, P), dtype=np.float32)   # [group, slot] dst codes
        zd[slot // P, slot % P] = dst_in_tile.astype(np.float32)
        NCH = GP // J
        zv = zv.reshape(NCH, J, P, IN_C).transpose(0, 2, 1, 3).reshape(NCH, P, J * IN_C)
        zd16 = zd.astype(np.float16).reshape(NCH, J, P).transpose(0, 2, 1)
        zd32 = np.ascontiguousarray(
            zd.reshape(NCH, J, P).transpose(0, 2, 1)).view(np.float16)  # [NCH,P,2J]
        z = np.ascontiguousarray(
            np.concatenate([zv, zd16, zd32], axis=2).transpose(1, 0, 2)
        ).reshape(P, GP * CW)

        xt = np.zeros((IN_C, NPAD), dtype=np.float16)
        xt[:, :NPC] = xf[c * NPC:(c + 1) * NPC].T
        cores_data.append({"z": z, "xt": xt})
    return cores_data, g_run, G, GP


def _build_program(g_run, GP, repeat=1, oh_pool_every=0):
    import concourse.tile as tile
    from concourse import bass, bacc, mybir
    from contextlib import ExitStack

    f32 = mybir.dt.float32
    f16 = mybir.dt.float16
    nc = bacc.Bacc("TRN2", target_bir_lowering=False, debug=False,
                   num_devices=NCORES)

    z_dram = nc.declare_dram_parameter("z", [P, GP * CW], f16, isOutput=False)
    xt_dram = nc.declare_dram_parameter("xt", [IN_C, NPAD], f16, isOutput=False)
    wgt_dram = nc.declare_dram_parameter("wgt", [NUM_BASES, IN_C * OUT_C], f32, isOutput=False)
    mct_dram = nc.declare_dram_parameter("mct", [NUM_BASES, R2], f32, isOutput=False)
    root_dram = nc.declare_dram_parameter("root", [IN_C, OUT_C], f16, isOutput=False)
    bias_dram = nc.declare_dram_parameter("bias", [OUT_C, 1], f32, isOutput=False)
    iota_dram = nc.declare_dram_parameter("iota", [P, OB * P], f16, isOutput=False)  # repeated iota
    out_dram = nc.declare_dram_parameter("out", [OUT_C, NPAD], f32, isOutput=True)

    w_scratch = nc.dram_tensor("w_scratch", [R2, IN_C * OUT_C], f32)

    g_cum = np.concatenate([[0], np.cumsum(g_run)])
    NPAIR = R2 // 2                       # 8 pairs -> 2 quads of 4

    with tile.TileContext(nc) as tc:
        with ExitStack() as ctx:
            const_p = ctx.enter_context(tc.tile_pool(name="const", bufs=1, space="SBUF"))
            zchunk_p = ctx.enter_context(tc.tile_pool(name="zchunk", bufs=4, space="SBUF"))
            oh_p = ctx.enter_context(tc.tile_pool(name="oh", bufs=4, space="SBUF"))
            agg_p = ctx.enter_context(tc.tile_pool(name="agg", bufs=3, space="SBUF"))
            out_p = ctx.enter_context(tc.tile_pool(name="outs", bufs=2, space="SBUF"))
            ps_quad_p = ctx.enter_context(tc.tile_pool(name="psquad", bufs=4, space="PSUM"))
            ps_out_p = ctx.enter_context(tc.tile_pool(name="psout", bufs=2, space="PSUM"))

            iota_t = const_p.tile([P, OB * P], f16)
            nc.sync.dma_start(out=iota_t[:], in_=iota_dram[:])
            iota1_t = const_p.tile([P, P], f16)   # plain 0..127 rows, for Pool
            nc.vector.tensor_copy(out=iota1_t[:], in_=iota_t[:, ::OB])
            root_t = const_p.tile([IN_C, OUT_C], f16)
            nc.sync.dma_start(out=root_t[:], in_=root_dram[:])
            bias_t = const_p.tile([OUT_C, 1], f32)
            nc.sync.dma_start(out=bias_t[:], in_=bias_dram[:])

            # ---- weight composition: W[r] = ((mask*comp) @ weight_flat)[r] ----
            mct_t = const_p.tile([NUM_BASES, R2], f32)
            nc.sync.dma_start(out=mct_t[:], in_=mct_dram[:])
            wgt_t = const_p.tile([NUM_BASES, IN_C * OUT_C], f32)
            nc.sync.dma_start(out=wgt_t[:], in_=wgt_dram[:])
            w_all = const_p.tile([R2, IN_C * OUT_C], f32)
            for k in range(IN_C * OUT_C // 512):
                ps_w = ps_quad_p.tile([R2, 512], f32, space="PSUM", name="ps_quad")
                nc.tensor.matmul(out=ps_w[:], lhsT=mct_t[:],
                                 rhs=wgt_t[:, k * 512:(k + 1) * 512],
                                 start=True, stop=True)
                nc.vector.tensor_copy(out=w_all[:, k * 512:(k + 1) * 512], in_=ps_w[:])
            nc.sync.dma_start(out=w_scratch[:, :], in_=w_all[:])
            w_pairs = []
            for pp in range(NPAIR):
                wp32 = const_p.tile([2 * IN_C, OUT_C], f32, name=f"wp32_{pp}")
                nc.sync.dma_start(out=wp32[:], in_=w_scratch[2 * pp:2 * pp + 2, :])
                wp16 = const_p.tile([2 * IN_C, OUT_C], f16, name=f"wp16_{pp}")
                nc.vector.tensor_copy(out=wp16[:], in_=wp32[:])
                w_pairs.append(wp16)

            # whole transposed own-x slab stays resident (fp16, 25KB/partition)
            xt_t = const_p.tile([IN_C, NPAD], f16)
            nc.sync.dma_start(out=xt_t[:], in_=xt_dram[:])

            # ---- main loop ----
            for rep in range(repeat):
                zt = None
                zt_ch = -1
                ohb = None
                ohb_id = -1
                ohb_pool = False
                o_sb = None
                nbld = 0
                for t in range(NTILES):
                    if t % 4 == 0:
                        o_sb = out_p.tile([OUT_C, 4 * P], f32, name="o_sb")
                    ps_out = ps_out_p.tile([OUT_C, P], f32, space="PSUM", name="ps_out")
                    pair_halves = []
                    for pair in range(NPAIR):
                        h = (int(g_run[t * R2 + 2 * pair]) > 0,
                             int(g_run[t * R2 + 2 * pair + 1]) > 0)
                        pair_halves.append(h)
                    n_mms = sum(1 for h in pair_halves if h[0] or h[1])
                    nc.tensor.matmul(out=ps_out[:], lhsT=root_t[:],
                                     rhs=xt_t[:, t * P:(t + 1) * P],
                                     start=True, stop=(n_mms == 0))
                    mm_i = 0
                    for q in range(2):
                        quad_pairs = [q * 4 + i for i in range(4)]
                        any_groups = any(pair_halves[pp][0] or pair_halves[pp][1]
                                         for pp in quad_pairs)
                        if not any_groups:
                            continue
                        ps_quad = ps_quad_p.tile([P, 4 * P], f32, space="PSUM",
                                                 name="ps_quad")
                        for qi, pair in enumerate(quad_pairs):
                            for half in range(2):
                                r = 2 * pair + half
                                kr = t * R2 + r
                                ng = int(g_run[kr])
                                if ng == 0:
                                    continue
                                g0 = int(g_cum[kr])
                                for k in range(ng):
                                    g = g0 + k
                                    ch, gl = g // J, g % J
                                    if ch != zt_ch:
                                        zt = zchunk_p.tile([P, J * CW], f16, name="zt")
                                        nc.sync.dma_start(
                                            out=zt[:],
                                            in_=z_dram[:, ch * J * CW:(ch + 1) * J * CW])
                                        zt_ch = ch
                                    bld = g // OB
                                    if bld != ohb_id:
                                        bl = (bld * OB) % J  # first group's gl
                                        pool_blk = (oh_pool_every and
                                                    nbld % oh_pool_every == 0)
                                        if pool_blk:
                                            ohb = oh_p.tile([P, OB * P], f16,
                                                            name="ohb")
                                            for j in range(OB):
                                                sc = zt[:, J * (IN_C + 1) + 2 * (bl + j):
                                                        J * (IN_C + 1) + 2 * (bl + j) + 2
                                                        ].bitcast(f32)
                                                nc.gpsimd.tensor_scalar(
                                                    out=ohb[:, j * P:(j + 1) * P],
                                                    in0=iota1_t[:], scalar1=sc,
                                                    scalar2=None,
                                                    op0=mybir.AluOpType.is_equal)
                                        else:
                                            ohb = oh_p.tile([P, OB * P], f16,
                                                            name="ohb")
                                            dstv = zt[:, J * IN_C + bl:
                                                      J * IN_C + bl + OB]
                                            nc.vector.tensor_tensor(
                                                out=ohb[:], in0=iota_t[:],
                                                in1=dstv.unsqueeze(1)
                                                        .broadcast_to([P, P, OB]),
                                                op=mybir.AluOpType.is_equal)
                                        ohb_pool = pool_blk
                                        ohb_id = bld
                                        nbld += 1
                                    rhs = (ohb[:, (g % OB) * P:(g % OB + 1) * P]
                                           if ohb_pool else ohb[:, (g % OB)::OB])
                                    nc.tensor.matmul(
                                        out=ps_quad[half * IN_C:(half + 1) * IN_C,
                                                    qi * P:(qi + 1) * P],
                                        lhsT=zt[:, gl * IN_C:(gl + 1) * IN_C],
                                        rhs=rhs,
                                        start=(k == 0), stop=(k == ng - 1))
                        agg_sb = agg_p.tile([P, 4 * P], f16, name="agg_sb")
                        nc.scalar.activation(
                            out=agg_sb[:], in_=ps_quad[:],
                            func=mybir.ActivationFunctionType.Copy)
                        for qi, pair in enumerate(quad_pairs):
                            h0, h1 = pair_halves[pair]
                            if not (h0 or h1):
                                continue
                            mm_i += 1
                            stop = (mm_i == n_mms)
                            if h0 and h1:
                                nc.tensor.matmul(
                                    out=ps_out[:], lhsT=w_pairs[pair],
                                    rhs=agg_sb[:, qi * P:(qi + 1) * P],
                                    start=False, stop=stop)
                            else:
                                half = 0 if h0 else 1
                                nc.tensor.matmul(
                                    out=ps_out[:],
                                    lhsT=w_pairs[pair][half * IN_C:(half + 1) * IN_C, :],
                                    rhs=agg_sb[half * IN_C:(half + 1) * IN_C,
                                               qi * P:(qi + 1) * P],
                                    start=False, stop=stop)
                    tq = t % 4
                    nc.scalar.activation(
                        out=o_sb[:, tq * P:(tq + 1) * P], in_=ps_out[:],
                        func=mybir.ActivationFunctionType.Identity,
                        bias=bias_t[:, 0:1])
                    if tq == 3 or t == NTILES - 1:
                        t0 = t - tq
                        nc.sync.dma_start(
                            out=out_dram[:, t0 * P:(t + 1) * P],
                            in_=o_sb[:, :(tq + 1) * P])

    nc.compile()
    return nc


def prepare(x, edge_index, edge_type, weight, comp, root, bias,
            repeat=1, oh_pool_every=5):
    x = np.asarray(x, dtype=np.float32)
    edge_index = np.asarray(edge_index)
    edge_type = np.asarray(edge_type)
    weight = np.asarray(weight, dtype=np.float32)
    comp = np.asarray(comp, dtype=np.float32)
    root = np.asarray(root, dtype=np.float32)
    bias = np.asarray(bias, dtype=np.float32)

    cores_data, g_run, G, GP = _host_prep(x, edge_index, edge_type)
    nc = _build_program(g_run, GP, repeat=repeat, oh_pool_every=oh_pool_every)

    mask = _build_weight_mask()
    mct = np.ascontiguousarray((mask * comp).T)          # [36, 16]
    wgt = weight.reshape(NUM_BASES, IN_C * OUT_C)
    iota = np.tile(np.repeat(np.arange(P, dtype=np.float16), OB)[None, :], (P, 1))
    bias_col = bias.reshape(OUT_C, 1)

    in_maps = []
    for c in range(NCORES):
        d = cores_data[c]
        in_maps.append({
            "z": d["z"], "xt": d["xt"], "wgt": wgt, "mct": mct,
            "root": root.astype(np.float16), "bias": bias_col, "iota": iota,
        })
    return nc, in_maps


def assemble(results):
    out = np.empty((N_NODES, OUT_C), dtype=np.float32)
    for c in range(NCORES):
        out[c * NPC:(c + 1) * NPC] = results[c]["out"][:, :NPC].T
    return out


def kernel(x, edge_index, edge_type, weight, comp, root, bias):
    from concourse.bass_utils import run_bass_kernel_spmd

    nc, in_maps = prepare(x, edge_index, edge_type, weight, comp, root, bias)
    res = run_bass_kernel_spmd(nc, in_maps, core_ids=list(range(NCORES)))
    return assemble(res.results)


# revision 38
# speedup vs baseline: 1.4211x; 1.0153x over previous
"""DRGCN message-passing kernel for 8 Trainium2 NeuronCores.

Strategy: shard by destination-node range (12500 nodes/core) so each core
computes its output rows fully locally (no collectives). Host preprocesses
edges into a padded, (tile, relation)-sorted slot layout, pre-gathers source
features (pre-scaled by the segment 1/count) into a streaming z layout.

Device inner loop per dst tile (128 nodes):
  - for each relation r (16), accumulate agg_r^T = sum_slots z_slot one-hot
    scatter matmuls in [in=64, dst=128] orientation: matmul(lhsT=z[slot,64],
    rhs=onehot[slot,dst]) -> PSUM. Two relations stack into one [128,128]
    PSUM region (partition halves), four pairs per PSUM bank ("quad").
  - one activation copy per quad PSUM->SBUF (fp16), then one matmul per
    relation-pair: lhsT=[W_r0;W_r1] [128,64], rhs=agg pair [128,128],
    accumulating into ps_out[64,128] on top of the root term.
  - one-hot matrices are built 4 groups at a time with a single DVE
    tensor_tensor(is_equal) against a tiled iota, using a stride-65 +
    broadcast access pattern on the dst-code column of the z stream.
"""
import numpy as np

N_NODES = 100000
IN_C = 64
OUT_C = 64
NUM_REL = 8
R2 = 2 * NUM_REL            # 16
NUM_M, NUM_N, NUM_O = 4, 2, 1
NUM_BASES = NUM_M + NUM_N * NUM_REL + NUM_O * R2  # 36
P = 128
NCORES = 8
NPC = N_NODES // NCORES     # 12500 nodes per core
NTILES = (NPC + P - 1) // P  # 98
NPAD = NTILES * P            # 12544
NRUNS = NTILES * R2          # 1568 runs per core
J = 32                       # groups per z-chunk DMA (multiple of OB)
OB = 16                      # one-hot build batch (groups per DVE instr)
CW = IN_C + 1                # 65 cols/group: 64 vals + dst f16


def _build_weight_mask():
    m = np.zeros((R2, NUM_BASES), dtype=np.float32)
    m[:, :NUM_M] = 1.0
    for row_i in range(R2):
        for col_i in range(NUM_REL):
            if row_i == col_i or row_i == col_i + NUM_REL:
                c = col_i * NUM_N
                m[row_i, NUM_M + c:NUM_M + c + NUM_N] = 1.0
        for col_i in range(R2):
            if row_i == col_i:
                s = NUM_M + NUM_N * NUM_REL + col_i * NUM_O
                m[row_i, s:s + NUM_O] = 1.0
    return m


def _host_prep(x, edge_index, edge_type):
    """Sort/pad edges per core, pre-gather scaled source features.

    Returns per-core dicts {z, xt} plus the shared group structure g_run, G.
    """
    src = np.concatenate([edge_index[0], edge_index[1]]).astype(np.int64)
    dst = np.concatenate([edge_index[1], edge_index[0]]).astype(np.int64)
    rel = np.concatenate([edge_type, edge_type + NUM_REL]).astype(np.int64)

    core = dst // NPC
    dst_local = dst - core * NPC
    key = (dst_local // P) * R2 + rel          # run id within core

    run_counts = np.zeros((NCORES, NRUNS), dtype=np.int64)
    per_core = []
    for c in range(NCORES):
        m = core == c
        s_c, dl_c, k_c = src[m], dst_local[m], key[m]
        order = np.argsort(k_c, kind="stable")
        s_c, dl_c, k_c = s_c[order], dl_c[order], k_c[order]
        run_counts[c] = np.bincount(k_c, minlength=NRUNS)
        per_core.append((s_c, dl_c, k_c))

    maxcnt = np.max(run_counts, axis=0)                  # shared across cores
    g_run = (maxcnt + P - 1) // P
    # split each run into (g_run-1) full groups + one tail segment; pack the
    # 16 tails of every tile into shared "bin" groups (first-fit, run order).
    full_f = np.maximum(g_run - 1, 0).astype(np.int64)
    tail_sz = (maxcnt - full_f * P).astype(np.int64)     # in (0, P]
    full_g0 = np.zeros(NRUNS, np.int64)
    tail_g = np.zeros(NRUNS, np.int64)
    tail_off = np.zeros(NRUNS, np.int64)
    # one accumulation chain per PSUM partition-row region may be open at a
    # time (matmul start=True lazily marks the full 2KB bank row pending-zero)
    # so each run's groups stay contiguous: fulls then its own tail group.
    gidx = 0
    for kr in range(NRUNS):
        full_g0[kr] = gidx
        gidx += int(full_f[kr])
        if tail_sz[kr] > 0:
            tail_g[kr] = gidx
            gidx += 1
        else:
            tail_g[kr] = -1
    G = int(gidx)
    GP = ((G + J - 1) // J) * J
    layout = dict(full_f=full_f, full_g0=full_g0, tail_g=tail_g,
                  tail_off=tail_off, tail_sz=tail_sz, G=G, GP=GP)

    xf = x.astype(np.float32)
    cores_data = []
    for c in range(NCORES):
        s_c, dl_c, k_c = per_core[c]
        cnt_c = run_counts[c]
        run_starts = np.concatenate([[0], np.cumsum(cnt_c)])[:-1]
        rank = np.arange(len(k_c)) - run_starts[k_c]
        nf = full_f[k_c] * P
        slot = np.where(
            rank < nf,
            full_g0[k_c] * P + rank,
            tail_g[k_c] * P + tail_off[k_c] + (rank - nf))   # global slot id
        dst_in_tile = dl_c - (k_c // R2) * P             # 0..127
        # per-(rel,dst) counts -> fold 1/cnt into the gathered features
        subkey = k_c * P + dst_in_tile
        cnt_edge = np.bincount(subkey, minlength=NRUNS * P)[subkey]
        vals = xf[s_c] * (1.0 / cnt_edge)[:, None].astype(np.float32)

        # chunk layout: [J groups x 64 value cols | J dst f16 | J dst f32]
        zv = np.zeros((GP * P, IN_C), dtype=np.float16)
        zv[slot, :] = vals.astype(np.float16)
        zd = np.zeros((GP, P), dtype=np.float32)   # [group, slot] dst codes
        zd[slot // P, slot % P] = dst_in_tile.astype(np.float32)
        NCH = GP // J
        zv = zv.reshape(NCH, J, P, IN_C).transpose(0, 2, 1, 3).reshape(NCH, P, J * IN_C)
        zd16 = zd.astype(np.float16).reshape(NCH, J, P).transpose(0, 2, 1)
        z = np.ascontiguousarray(
            np.concatenate([zv, zd16], axis=2).transpose(1, 0, 2)
        ).reshape(P, GP * CW)

        xt = np.zeros((IN_C, NPAD), dtype=np.float16)
        xt[:, :NPC] = xf[c * NPC:(c + 1) * NPC].T
        cores_data.append({"z": z, "xt": xt})
    return cores_data, layout


def _build_program(layout, repeat=1, oh_pool_every=0, dbg_const_oh=False,
                   dbg_no_mm=False):
    import concourse.tile as tile
    from concourse import bass, bacc, mybir
    from contextlib import ExitStack

    f32 = mybir.dt.float32
    f16 = mybir.dt.float16
    GP = layout["GP"]
    nc = bacc.Bacc("TRN2", target_bir_lowering=False, debug=False,
                   num_devices=NCORES)

    z_dram = nc.declare_dram_parameter("z", [P, GP * CW], f16, isOutput=False)
    xt_dram = nc.declare_dram_parameter("xt", [IN_C, NPAD], f16, isOutput=False)
    wgt_dram = nc.declare_dram_parameter("wgt", [NUM_BASES, IN_C * OUT_C], f32, isOutput=False)
    mct_dram = nc.declare_dram_parameter("mct", [NUM_BASES, R2], f32, isOutput=False)
    root_dram = nc.declare_dram_parameter("root", [IN_C, OUT_C], f16, isOutput=False)
    bias_dram = nc.declare_dram_parameter("bias", [OUT_C, 1], f32, isOutput=False)
    iota_dram = nc.declare_dram_parameter("iota", [P, OB * P], f16, isOutput=False)  # repeated iota
    out_dram = nc.declare_dram_parameter("out", [OUT_C, NPAD], f32, isOutput=True)

    w_scratch = nc.dram_tensor("w_scratch", [R2, IN_C * OUT_C], f32)

    full_f, full_g0 = layout["full_f"], layout["full_g0"]
    tail_g, tail_off, tail_sz = layout["tail_g"], layout["tail_off"], layout["tail_sz"]
    NPAIR = R2 // 2                       # 8 pairs -> 2 quads of 4

    with tile.TileContext(nc) as tc:
        with ExitStack() as ctx:
            const_p = ctx.enter_context(tc.tile_pool(name="const", bufs=1, space="SBUF"))
            zchunk_p = ctx.enter_context(tc.tile_pool(name="zchunk", bufs=4, space="SBUF"))
            oh_p = ctx.enter_context(tc.tile_pool(name="oh", bufs=4, space="SBUF"))
            agg_p = ctx.enter_context(tc.tile_pool(name="agg", bufs=3, space="SBUF"))
            out_p = ctx.enter_context(tc.tile_pool(name="outs", bufs=2, space="SBUF"))
            ps_quad_p = ctx.enter_context(tc.tile_pool(name="psquad", bufs=4, space="PSUM"))
            ps_out_p = ctx.enter_context(tc.tile_pool(name="psout", bufs=2, space="PSUM"))

            iota_t = const_p.tile([P, OB * P], f16)
            nc.sync.dma_start(out=iota_t[:], in_=iota_dram[:])
            iota1_t = const_p.tile([P, P], f16)   # plain 0..127 rows, for Pool
            nc.vector.tensor_copy(out=iota1_t[:], in_=iota_t[:, ::OB])
            root_t = const_p.tile([IN_C, OUT_C], f16)
            nc.sync.dma_start(out=root_t[:], in_=root_dram[:])
            bias_t = const_p.tile([OUT_C, 1], f32)
            nc.sync.dma_start(out=bias_t[:], in_=bias_dram[:])

            # ---- weight composition: W[r] = ((mask*comp) @ weight_flat)[r] ----
            mct_t = const_p.tile([NUM_BASES, R2], f32)
            nc.sync.dma_start(out=mct_t[:], in_=mct_dram[:])
            wgt_t = const_p.tile([NUM_BASES, IN_C * OUT_C], f32)
            nc.sync.dma_start(out=wgt_t[:], in_=wgt_dram[:])
            w_all = const_p.tile([R2, IN_C * OUT_C], f32)
            for k in range(IN_C * OUT_C // 512):
                ps_w = ps_quad_p.tile([R2, 512], f32, space="PSUM", name="ps_quad")
                nc.tensor.matmul(out=ps_w[:], lhsT=mct_t[:],
                                 rhs=wgt_t[:, k * 512:(k + 1) * 512],
                                 start=True, stop=True)
                nc.vector.tensor_copy(out=w_all[:, k * 512:(k + 1) * 512], in_=ps_w[:])
            nc.sync.dma_start(out=w_scratch[:, :], in_=w_all[:])
            w_pairs = []
            for pp in range(NPAIR):
                wp32 = const_p.tile([2 * IN_C, OUT_C], f32, name=f"wp32_{pp}")
                nc.sync.dma_start(out=wp32[:], in_=w_scratch[2 * pp:2 * pp + 2, :])
                wp16 = const_p.tile([2 * IN_C, OUT_C], f16, name=f"wp16_{pp}")
                nc.vector.tensor_copy(out=wp16[:], in_=wp32[:])
                w_pairs.append(wp16)

            # whole transposed own-x slab stays resident (fp16, 25KB/partition)
            xt_t = const_p.tile([IN_C, NPAD], f16)
            nc.sync.dma_start(out=xt_t[:], in_=xt_dram[:])

            oh_const = None
            if dbg_const_oh:  # timing diagnostics only: skip one-hot builds
                oh_const = const_p.tile([P, OB * P], f16)
                nc.vector.tensor_copy(out=oh_const[:], in_=iota_t[:])

            # ---- main loop ----
            for rep in range(repeat):
                zt = None
                zt_ch = -1
                ohb = None
                ohb_id = -1
                ohb_pool = False
                o_sb = None
                nbld = 0
                def touch_group(g):
                    """Ensure chunk DMA + one-hot build for group g; return
                    (zt, gl, oh_ap) where oh_ap is the [P, P]-col view."""
                    nonlocal zt, zt_ch, ohb, ohb_id, nbld
                    ch, gl = g // J, g % J
                    if ch != zt_ch:
                        zt = zchunk_p.tile([P, J * CW], f16, name="zt")
                        nc.sync.dma_start(
                            out=zt[:],
                            in_=z_dram[:, ch * J * CW:(ch + 1) * J * CW])
                        zt_ch = ch
                    bld = g // OB
                    if dbg_const_oh:
                        return zt, gl, oh_const[:, 0::OB]
                    if bld != ohb_id:
                        bl = (bld * OB) % J  # first group's gl
                        ohb = oh_p.tile([P, OB * P], f16, name="ohb")
                        dstv = zt[:, J * IN_C + bl:J * IN_C + bl + OB]
                        nc.vector.tensor_tensor(
                            out=ohb[:], in0=iota_t[:],
                            in1=dstv.unsqueeze(1).broadcast_to([P, P, OB]),
                            op=mybir.AluOpType.is_equal)
                        ohb_id = bld
                        nbld += 1
                    return zt, gl, ohb[:, (g % OB)::OB]

                for t in range(NTILES):
                    if t % 4 == 0:
                        o_sb = out_p.tile([OUT_C, 4 * P], f32, name="o_sb")
                    ps_out = ps_out_p.tile([OUT_C, P], f32, space="PSUM", name="ps_out")
                    has_r = [int(full_f[t * R2 + r]) > 0 or int(tail_sz[t * R2 + r]) > 0
                             for r in range(R2)]
                    n_mms = sum(1 for pair in range(NPAIR)
                                if has_r[2 * pair] or has_r[2 * pair + 1])
                    nc.tensor.matmul(out=ps_out[:], lhsT=root_t[:],
                                     rhs=xt_t[:, t * P:(t + 1) * P],
                                     start=True, stop=(n_mms == 0))
                    ps_quads = [ps_quad_p.tile([P, 4 * P], f32, space="PSUM",
                                               name="ps_quad") for _ in range(2)]

                    def region(r, ps_quads=ps_quads):
                        half, pr = r % 2, r // 2
                        return ps_quads[pr // 4][half * IN_C:(half + 1) * IN_C,
                                                 (pr % 4) * P:(pr % 4 + 1) * P]

                    # sequential chain per run: fulls then own tail group
                    for r in range(R2):
                        kr = t * R2 + r
                        for k in range(int(full_f[kr])):
                            ztl, gl, oh_ap = touch_group(int(full_g0[kr]) + k)
                            nc.tensor.matmul(
                                out=region(r),
                                lhsT=ztl[:, gl * IN_C:(gl + 1) * IN_C],
                                rhs=oh_ap,
                                start=(k == 0), stop=False)
                        s = int(tail_sz[kr])
                        if s == 0:
                            continue
                        a = int(tail_off[kr])
                        ztl, gl, oh_ap = touch_group(int(tail_g[kr]))
                        nc.tensor.matmul(
                            out=region(r),
                            lhsT=ztl[a:a + s, gl * IN_C:(gl + 1) * IN_C],
                            rhs=oh_ap[a:a + s, :],
                            start=(int(full_f[kr]) == 0), stop=True)
                    # tails done: copy quads, accumulate W matmuls
                    mm_i = 0
                    for q in range(2):
                        quad_pairs = [q * 4 + i for i in range(4)]
                        if not any(has_r[2 * pp] or has_r[2 * pp + 1]
                                   for pp in quad_pairs):
                            continue
                        agg_sb = agg_p.tile([P, 4 * P], f16, name="agg_sb")
                        nc.scalar.activation(
                            out=agg_sb[:], in_=ps_quads[q][:],
                            func=mybir.ActivationFunctionType.Copy)
                        for qi, pair in enumerate(quad_pairs):
                            h0, h1 = has_r[2 * pair], has_r[2 * pair + 1]
                            if not (h0 or h1):
                                continue
                            mm_i += 1
                            stop = (mm_i == n_mms)
                            if h0 and h1:
                                nc.tensor.matmul(
                                    out=ps_out[:], lhsT=w_pairs[pair],
                                    rhs=agg_sb[:, qi * P:(qi + 1) * P],
                                    start=False, stop=stop)
                            else:
                                half = 0 if h0 else 1
                                nc.tensor.matmul(
                                    out=ps_out[:],
                                    lhsT=w_pairs[pair][half * IN_C:(half + 1) * IN_C, :],
                                    rhs=agg_sb[half * IN_C:(half + 1) * IN_C,
                                               qi * P:(qi + 1) * P],
                                    start=False, stop=stop)
                    tq = t % 4
                    nc.scalar.activation(
                        out=o_sb[:, tq * P:(tq + 1) * P], in_=ps_out[:],
                        func=mybir.ActivationFunctionType.Identity,
                        bias=bias_t[:, 0:1])
                    if tq == 3 or t == NTILES - 1:
                        t0 = t - tq
                        nc.sync.dma_start(
                            out=out_dram[:, t0 * P:(t + 1) * P],
                            in_=o_sb[:, :(tq + 1) * P])

    nc.compile()
    return nc


def prepare(x, edge_index, edge_type, weight, comp, root, bias,
            repeat=1, oh_pool_every=0, dbg_const_oh=False, dbg_no_mm=False):
    x = np.asarray(x, dtype=np.float32)
    edge_index = np.asarray(edge_index)
    edge_type = np.asarray(edge_type)
    weight = np.asarray(weight, dtype=np.float32)
    comp = np.asarray(comp, dtype=np.float32)
    root = np.asarray(root, dtype=np.float32)
    bias = np.asarray(bias, dtype=np.float32)

    cores_data, layout = _host_prep(x, edge_index, edge_type)
    nc = _build_program(layout, repeat=repeat, oh_pool_every=oh_pool_every,
                        dbg_const_oh=dbg_const_oh, dbg_no_mm=dbg_no_mm)

    mask = _build_weight_mask()
    mct = np.ascontiguousarray((mask * comp).T)          # [36, 16]
    wgt = weight.reshape(NUM_BASES, IN_C * OUT_C)
    iota = np.tile(np.repeat(np.arange(P, dtype=np.float16), OB)[None, :], (P, 1))
    bias_col = bias.reshape(OUT_C, 1)

    in_maps = []
    for c in range(NCORES):
        d = cores_data[c]
        in_maps.append({
            "z": d["z"], "xt": d["xt"], "wgt": wgt, "mct": mct,
            "root": root.astype(np.float16), "bias": bias_col, "iota": iota,
        })
    return nc, in_maps


def assemble(results):
    out = np.empty((N_NODES, OUT_C), dtype=np.float32)
    for c in range(NCORES):
        out[c * NPC:(c + 1) * NPC] = results[c]["out"][:, :NPC].T
    return out


def kernel(x, edge_index, edge_type, weight, comp, root, bias):
    from concourse.bass_utils import run_bass_kernel_spmd

    nc, in_maps = prepare(x, edge_index, edge_type, weight, comp, root, bias)
    res = run_bass_kernel_spmd(nc, in_maps, core_ids=list(range(NCORES)))
    return assemble(res.results)


# revision 39
# speedup vs baseline: 206.2184x; 145.1153x over previous
"""DRGCN message-passing kernel for 8 Trainium2 NeuronCores.

Strategy: shard by destination-node range (12500 nodes/core) so each core
computes its output rows fully locally (no collectives). Host preprocesses
edges into a padded, (tile, relation)-sorted slot layout, pre-gathers source
features (pre-scaled by the segment 1/count) into a streaming z layout.

Device inner loop per dst tile (128 nodes):
  - for each relation r (16), accumulate agg_r^T = sum_slots z_slot one-hot
    scatter matmuls in [in=64, dst=128] orientation: matmul(lhsT=z[slot,64],
    rhs=onehot[slot,dst]) -> PSUM. Two relations stack into one [128,128]
    PSUM region (partition halves), four pairs per PSUM bank ("quad").
  - one activation copy per quad PSUM->SBUF (fp16), then one matmul per
    relation-pair: lhsT=[W_r0;W_r1] [128,64], rhs=agg pair [128,128],
    accumulating into ps_out[64,128] on top of the root term.
  - one-hot matrices are built 4 groups at a time with a single DVE
    tensor_tensor(is_equal) against a tiled iota, using a stride-65 +
    broadcast access pattern on the dst-code column of the z stream.
"""
import numpy as np

N_NODES = 100000
IN_C = 64
OUT_C = 64
NUM_REL = 8
R2 = 2 * NUM_REL            # 16
NUM_M, NUM_N, NUM_O = 4, 2, 1
NUM_BASES = NUM_M + NUM_N * NUM_REL + NUM_O * R2  # 36
P = 128
NCORES = 8
NPC = N_NODES // NCORES     # 12500 nodes per core
NTILES = (NPC + P - 1) // P  # 98
NPAD = NTILES * P            # 12544
NRUNS = NTILES * R2          # 1568 runs per core
J = 32                       # groups per z-chunk DMA (multiple of OB)
OB = 8                       # one-hot build batch (groups per DVE instr)
CW = IN_C + 1                # 65 cols/group: 64 vals + dst f16


def _build_weight_mask():
    m = np.zeros((R2, NUM_BASES), dtype=np.float32)
    m[:, :NUM_M] = 1.0
    for row_i in range(R2):
        for col_i in range(NUM_REL):
            if row_i == col_i or row_i == col_i + NUM_REL:
                c = col_i * NUM_N
                m[row_i, NUM_M + c:NUM_M + c + NUM_N] = 1.0
        for col_i in range(R2):
            if row_i == col_i:
                s = NUM_M + NUM_N * NUM_REL + col_i * NUM_O
                m[row_i, s:s + NUM_O] = 1.0
    return m


def _host_prep(x, edge_index, edge_type):
    """Sort/pad edges per core, pre-gather scaled source features.

    Returns per-core dicts {z, xt} plus the shared group structure g_run, G.
    """
    src = np.concatenate([edge_index[0], edge_index[1]]).astype(np.int64)
    dst = np.concatenate([edge_index[1], edge_index[0]]).astype(np.int64)
    rel = np.concatenate([edge_type, edge_type + NUM_REL]).astype(np.int64)

    core = dst // NPC
    dst_local = dst - core * NPC
    key = (dst_local // P) * R2 + rel          # run id within core

    run_counts = np.zeros((NCORES, NRUNS), dtype=np.int64)
    per_core = []
    for c in range(NCORES):
        m = core == c
        s_c, dl_c, k_c = src[m], dst_local[m], key[m]
        order = np.argsort(k_c, kind="stable")
        s_c, dl_c, k_c = s_c[order], dl_c[order], k_c[order]
        run_counts[c] = np.bincount(k_c, minlength=NRUNS)
        per_core.append((s_c, dl_c, k_c))

    maxcnt = np.max(run_counts, axis=0)                  # shared across cores
    g_run = (maxcnt + P - 1) // P
    # split each run into (g_run-1) full groups + one tail segment; pack the
    # 16 tails of every tile into shared "bin" groups (first-fit, run order).
    full_f = np.maximum(g_run - 1, 0).astype(np.int64)
    tail_sz = (maxcnt - full_f * P).astype(np.int64)     # in (0, P]
    full_g0 = np.zeros(NRUNS, np.int64)
    tail_g = np.zeros(NRUNS, np.int64)
    tail_off = np.zeros(NRUNS, np.int64)
    # one accumulation chain per PSUM partition-row region may be open at a
    # time (matmul start=True lazily marks the full 2KB bank row pending-zero)
    # so each run's groups stay contiguous: fulls then its own tail group.
    gidx = 0
    for kr in range(NRUNS):
        full_g0[kr] = gidx
        gidx += int(full_f[kr])
        if tail_sz[kr] > 0:
            tail_g[kr] = gidx
            gidx += 1
        else:
            tail_g[kr] = -1
    G = int(gidx)
    GP = ((G + J - 1) // J) * J
    layout = dict(full_f=full_f, full_g0=full_g0, tail_g=tail_g,
                  tail_off=tail_off, tail_sz=tail_sz, G=G, GP=GP)

    xf = x.astype(np.float32)
    cores_data = []
    for c in range(NCORES):
        s_c, dl_c, k_c = per_core[c]
        cnt_c = run_counts[c]
        run_starts = np.concatenate([[0], np.cumsum(cnt_c)])[:-1]
        rank = np.arange(len(k_c)) - run_starts[k_c]
        nf = full_f[k_c] * P
        slot = np.where(
            rank < nf,
            full_g0[k_c] * P + rank,
            tail_g[k_c] * P + tail_off[k_c] + (rank - nf))   # global slot id
        dst_in_tile = dl_c - (k_c // R2) * P             # 0..127
        # per-(rel,dst) counts -> fold 1/cnt into the gathered features
        subkey = k_c * P + dst_in_tile
        cnt_edge = np.bincount(subkey, minlength=NRUNS * P)[subkey]
        vals = xf[s_c] * (1.0 / cnt_edge)[:, None].astype(np.float32)

        # chunk layout: [J groups x 64 value cols | J dst f16 | J dst f32]
        zv = np.zeros((GP * P, IN_C), dtype=np.float16)
        zv[slot, :] = vals.astype(np.float16)
        zd = np.zeros((GP, P), dtype=np.float32)   # [group, slot] dst codes
        zd[slot // P, slot % P] = dst_in_tile.astype(np.float32)
        NCH = GP // J
        zv = zv.reshape(NCH, J, P, IN_C).transpose(0, 2, 1, 3).reshape(NCH, P, J * IN_C)
        zd16 = zd.astype(np.float16).reshape(NCH, J, P).transpose(0, 2, 1)
        z = np.ascontiguousarray(
            np.concatenate([zv, zd16], axis=2).transpose(1, 0, 2)
        ).reshape(P, GP * CW)

        xt = np.zeros((IN_C, NPAD), dtype=np.float16)
        xt[:, :NPC] = xf[c * NPC:(c + 1) * NPC].T
        cores_data.append({"z": z, "xt": xt})
    return cores_data, layout


def _build_program(layout, repeat=1, oh_pool_every=0, dbg_const_oh=False,
                   dbg_no_mm=False):
    import concourse.tile as tile
    from concourse import bass, bacc, mybir
    from contextlib import ExitStack

    f32 = mybir.dt.float32
    f16 = mybir.dt.float16
    GP = layout["GP"]
    nc = bacc.Bacc("TRN2", target_bir_lowering=False, debug=False,
                   num_devices=NCORES)

    z_dram = nc.declare_dram_parameter("z", [P, GP * CW], f16, isOutput=False)
    xt_dram = nc.declare_dram_parameter("xt", [IN_C, NPAD], f16, isOutput=False)
    wgt_dram = nc.declare_dram_parameter("wgt", [NUM_BASES, IN_C * OUT_C], f32, isOutput=False)
    mct_dram = nc.declare_dram_parameter("mct", [NUM_BASES, R2], f32, isOutput=False)
    root_dram = nc.declare_dram_parameter("root", [IN_C, OUT_C], f16, isOutput=False)
    bias_dram = nc.declare_dram_parameter("bias", [OUT_C, 1], f32, isOutput=False)
    iota_dram = nc.declare_dram_parameter("iota", [P, OB * P], f16, isOutput=False)  # repeated iota
    out_dram = nc.declare_dram_parameter("out", [OUT_C, NPAD], f32, isOutput=True)

    w_scratch = nc.dram_tensor("w_scratch", [R2, IN_C * OUT_C], f32)

    full_f, full_g0 = layout["full_f"], layout["full_g0"]
    tail_g, tail_off, tail_sz = layout["tail_g"], layout["tail_off"], layout["tail_sz"]
    NPAIR = R2 // 2                       # 8 pairs -> 2 quads of 4

    with tile.TileContext(nc) as tc:
        with ExitStack() as ctx:
            const_p = ctx.enter_context(tc.tile_pool(name="const", bufs=1, space="SBUF"))
            zchunk_p = ctx.enter_context(tc.tile_pool(name="zchunk", bufs=4, space="SBUF"))
            oh_p = ctx.enter_context(tc.tile_pool(name="oh", bufs=4, space="SBUF"))
            agg_p = ctx.enter_context(tc.tile_pool(name="agg", bufs=3, space="SBUF"))
            out_p = ctx.enter_context(tc.tile_pool(name="outs", bufs=2, space="SBUF"))
            ps_quad_p = ctx.enter_context(tc.tile_pool(name="psquad", bufs=4, space="PSUM"))
            ps_out_p = ctx.enter_context(tc.tile_pool(name="psout", bufs=2, space="PSUM"))

            iota_t = const_p.tile([P, OB * P], f16)
            nc.sync.dma_start(out=iota_t[:], in_=iota_dram[:])
            iota1_t = const_p.tile([P, P], f16)   # plain 0..127 rows, for Pool
            nc.vector.tensor_copy(out=iota1_t[:], in_=iota_t[:, ::OB])
            root_t = const_p.tile([IN_C, OUT_C], f16)
            nc.sync.dma_start(out=root_t[:], in_=root_dram[:])
            bias_t = const_p.tile([OUT_C, 1], f32)
            nc.sync.dma_start(out=bias_t[:], in_=bias_dram[:])

            # ---- weight composition: W[r] = ((mask*comp) @ weight_flat)[r] ----
            mct_t = const_p.tile([NUM_BASES, R2], f32)
            nc.sync.dma_start(out=mct_t[:], in_=mct_dram[:])
            wgt_t = const_p.tile([NUM_BASES, IN_C * OUT_C], f32)
            nc.sync.dma_start(out=wgt_t[:], in_=wgt_dram[:])
            w_all = const_p.tile([R2, IN_C * OUT_C], f32)
            for k in range(IN_C * OUT_C // 512):
                ps_w = ps_quad_p.tile([R2, 512], f32, space="PSUM", name="ps_quad")
                nc.tensor.matmul(out=ps_w[:], lhsT=mct_t[:],
                                 rhs=wgt_t[:, k * 512:(k + 1) * 512],
                                 start=True, stop=True)
                nc.vector.tensor_copy(out=w_all[:, k * 512:(k + 1) * 512], in_=ps_w[:])
            nc.sync.dma_start(out=w_scratch[:, :], in_=w_all[:])
            w_pairs = []
            for pp in range(NPAIR):
                wp32 = const_p.tile([2 * IN_C, OUT_C], f32, name=f"wp32_{pp}")
                nc.sync.dma_start(out=wp32[:], in_=w_scratch[2 * pp:2 * pp + 2, :])
                wp16 = const_p.tile([2 * IN_C, OUT_C], f16, name=f"wp16_{pp}")
                nc.vector.tensor_copy(out=wp16[:], in_=wp32[:])
                w_pairs.append(wp16)

            # whole transposed own-x slab stays resident (fp16, 25KB/partition)
            xt_t = const_p.tile([IN_C, NPAD], f16)
            nc.sync.dma_start(out=xt_t[:], in_=xt_dram[:])

            oh_const = None
            if dbg_const_oh:  # timing diagnostics only: skip one-hot builds
                oh_const = const_p.tile([P, OB * P], f16)
                nc.vector.tensor_copy(out=oh_const[:], in_=iota_t[:])

            # ---- main loop ----
            for rep in range(repeat):
                zt = None
                zt_ch = -1
                ohb = None
                ohb_id = -1
                ohb_pool = False
                o_sb = None
                nbld = 0
                def touch_group(g):
                    """Ensure chunk DMA + one-hot build for group g; return
                    (zt, gl, oh_ap) where oh_ap is the [P, P]-col view."""
                    nonlocal zt, zt_ch, ohb, ohb_id, nbld
                    ch, gl = g // J, g % J
                    if ch != zt_ch:
                        zt = zchunk_p.tile([P, J * CW], f16, name="zt")
                        nc.sync.dma_start(
                            out=zt[:],
                            in_=z_dram[:, ch * J * CW:(ch + 1) * J * CW])
                        zt_ch = ch
                    bld = g // OB
                    if dbg_const_oh:
                        return zt, gl, oh_const[:, 0::OB]
                    if bld != ohb_id:
                        bl = (bld * OB) % J  # first group's gl
                        ohb = oh_p.tile([P, OB * P], f16, name="ohb")
                        dstv = zt[:, J * IN_C + bl:J * IN_C + bl + OB]
                        nc.vector.tensor_tensor(
                            out=ohb[:], in0=iota_t[:],
                            in1=dstv.unsqueeze(1).broadcast_to([P, P, OB]),
                            op=mybir.AluOpType.is_equal)
                        ohb_id = bld
                        nbld += 1
                    return zt, gl, ohb[:, (g % OB)::OB]

                for t in range(NTILES):
                    if t % 4 == 0:
                        o_sb = out_p.tile([OUT_C, 4 * P], f32, name="o_sb")
                    ps_out = ps_out_p.tile([OUT_C, P], f32, space="PSUM", name="ps_out")
                    has_r = [int(full_f[t * R2 + r]) > 0 or int(tail_sz[t * R2 + r]) > 0
                             for r in range(R2)]
                    n_mms = sum(1 for pair in range(NPAIR)
                                if has_r[2 * pair] or has_r[2 * pair + 1])
                    nc.tensor.matmul(out=ps_out[:], lhsT=root_t[:],
                                     rhs=xt_t[:, t * P:(t + 1) * P],
                                     start=True, stop=(n_mms == 0))
                    ps_quads = [ps_quad_p.tile([P, 4 * P], f32, space="PSUM",
                                               name="ps_quad") for _ in range(2)]

                    def region(r, ps_quads=ps_quads):
                        half, pr = r % 2, r // 2
                        return ps_quads[pr // 4][half * IN_C:(half + 1) * IN_C,
                                                 (pr % 4) * P:(pr % 4 + 1) * P]

                    # sequential chain per run: fulls then own tail group
                    for r in range(R2):
                        kr = t * R2 + r
                        for k in range(int(full_f[kr])):
                            ztl, gl, oh_ap = touch_group(int(full_g0[kr]) + k)
                            nc.tensor.matmul(
                                out=region(r),
                                lhsT=ztl[:, gl * IN_C:(gl + 1) * IN_C],
                                rhs=oh_ap,
                                start=(k == 0), stop=False)
                        s = int(tail_sz[kr])
                        if s == 0:
                            continue
                        a = int(tail_off[kr])
                        ztl, gl, oh_ap = touch_group(int(tail_g[kr]))
                        nc.tensor.matmul(
                            out=region(r),
                            lhsT=ztl[a:a + s, gl * IN_C:(gl + 1) * IN_C],
                            rhs=oh_ap[a:a + s, :],
                            start=(int(full_f[kr]) == 0), stop=True)
                    # tails done: copy quads, accumulate W matmuls
                    mm_i = 0
                    for q in range(2):
                        quad_pairs = [q * 4 + i for i in range(4)]
                        if not any(has_r[2 * pp] or has_r[2 * pp + 1]
                                   for pp in quad_pairs):
                            continue
                        agg_sb = agg_p.tile([P, 4 * P], f16, name="agg_sb")
                        nc.scalar.activation(
                            out=agg_sb[:], in_=ps_quads[q][:],
                            func=mybir.ActivationFunctionType.Copy)
                        for qi, pair in enumerate(quad_pairs):
                            h0, h1 = has_r[2 * pair], has_r[2 * pair + 1]
                            if not (h0 or h1):
                                continue
                            mm_i += 1
                            stop = (mm_i == n_mms)
                            if h0 and h1:
                                nc.tensor.matmul(
                                    out=ps_out[:], lhsT=w_pairs[pair],
                                    rhs=agg_sb[:, qi * P:(qi + 1) * P],
                                    start=False, stop=stop)
                            else:
                                half = 0 if h0 else 1
                                nc.tensor.matmul(
                                    out=ps_out[:],
                                    lhsT=w_pairs[pair][half * IN_C:(half + 1) * IN_C, :],
                                    rhs=agg_sb[half * IN_C:(half + 1) * IN_C,
                                               qi * P:(qi + 1) * P],
                                    start=False, stop=stop)
                    tq = t % 4
                    nc.scalar.activation(
                        out=o_sb[:, tq * P:(tq + 1) * P], in_=ps_out[:],
                        func=mybir.ActivationFunctionType.Identity,
                        bias=bias_t[:, 0:1])
                    if tq == 3 or t == NTILES - 1:
                        t0 = t - tq
                        nc.sync.dma_start(
                            out=out_dram[:, t0 * P:(t + 1) * P],
                            in_=o_sb[:, :(tq + 1) * P])

    nc.compile()
    return nc


def prepare(x, edge_index, edge_type, weight, comp, root, bias,
            repeat=1, oh_pool_every=0, dbg_const_oh=False, dbg_no_mm=False):
    x = np.asarray(x, dtype=np.float32)
    edge_index = np.asarray(edge_index)
    edge_type = np.asarray(edge_type)
    weight = np.asarray(weight, dtype=np.float32)
    comp = np.asarray(comp, dtype=np.float32)
    root = np.asarray(root, dtype=np.float32)
    bias = np.asarray(bias, dtype=np.float32)

    cores_data, layout = _host_prep(x, edge_index, edge_type)
    nc = _build_program(layout, repeat=repeat, oh_pool_every=oh_pool_every,
                        dbg_const_oh=dbg_const_oh, dbg_no_mm=dbg_no_mm)

    mask = _build_weight_mask()
    mct = np.ascontiguousarray((mask * comp).T)          # [36, 16]
    wgt = weight.reshape(NUM_BASES, IN_C * OUT_C)
    iota = np.tile(np.repeat(np.arange(P, dtype=np.float16), OB)[None, :], (P, 1))
    bias_col = bias.reshape(OUT_C, 1)

    in_maps = []
    for c in range(NCORES):
        d = cores_data[c]
        in_maps.append({
            "z": d["z"], "xt": d["xt"], "wgt": wgt, "mct": mct,
            "root": root.astype(np.float16), "bias": bias_col, "iota": iota,
        })
    return nc, in_maps


def assemble(results):
    out = np.empty((N_NODES, OUT_C), dtype=np.float32)
    for c in range(NCORES):
        out[c * NPC:(c + 1) * NPC] = results[c]["out"][:, :NPC].T
    return out


def kernel(x, edge_index, edge_type, weight, comp, root, bias):
    from concourse.bass_utils import run_bass_kernel_spmd

    nc, in_maps = prepare(x, edge_index, edge_type, weight, comp, root, bias)
    res = run_bass_kernel_spmd(nc, in_maps, core_ids=list(range(NCORES)))
    return assemble(res.results)


# revision 45
# speedup vs baseline: 241.8551x; 1.1728x over previous
"""DRGCN message-passing kernel for 8 Trainium2 NeuronCores.

Strategy: shard by destination-node range (12500 nodes/core) so each core
computes its output rows fully locally (no collectives). Host preprocesses
edges into a padded, (tile, relation)-sorted slot layout, pre-gathers source
features (pre-scaled by the segment 1/count) into a streaming z layout.

Device inner loop per dst tile (128 nodes):
  - for each relation r (16), accumulate agg_r^T = sum_slots z_slot one-hot
    scatter matmuls in [in=64, dst=128] orientation: matmul(lhsT=z[slot,64],
    rhs=onehot[slot,dst]) -> PSUM. Two relations stack into one [128,128]
    PSUM region (partition halves), four pairs per PSUM bank ("quad").
  - one activation copy per quad PSUM->SBUF (fp16), then one matmul per
    relation-pair: lhsT=[W_r0;W_r1] [128,64], rhs=agg pair [128,128],
    accumulating into ps_out[64,128] on top of the root term.
  - one-hot matrices are built OB=8 groups at a time with a single DVE
    tensor_tensor(is_equal) against a repeated iota, using a broadcast
    access pattern over the dst-code columns of the z chunk; the matmul
    rhs reads the interleaved one-hot block with an OB-strided slice.
"""
import numpy as np

N_NODES = 100000
IN_C = 64
OUT_C = 64
NUM_REL = 8
R2 = 2 * NUM_REL            # 16
NUM_M, NUM_N, NUM_O = 4, 2, 1
NUM_BASES = NUM_M + NUM_N * NUM_REL + NUM_O * R2  # 36
P = 128
NCORES = 8
NPC = N_NODES // NCORES     # 12500 nodes per core
NTILES = (NPC + P - 1) // P  # 98
NPAD = NTILES * P            # 12544
NRUNS = NTILES * R2          # 1568 runs per core
J = 32                       # groups per z-chunk DMA (multiple of OB)
OB = 8                       # one-hot build batch (groups per DVE instr)
CW = IN_C + 1                # 65 cols/group: 64 vals + dst f16


def _build_weight_mask():
    m = np.zeros((R2, NUM_BASES), dtype=np.float32)
    m[:, :NUM_M] = 1.0
    for row_i in range(R2):
        for col_i in range(NUM_REL):
            if row_i == col_i or row_i == col_i + NUM_REL:
                c = col_i * NUM_N
                m[row_i, NUM_M + c:NUM_M + c + NUM_N] = 1.0
        for col_i in range(R2):
            if row_i == col_i:
                s = NUM_M + NUM_N * NUM_REL + col_i * NUM_O
                m[row_i, s:s + NUM_O] = 1.0
    return m


def _host_prep(x, edge_index, edge_type):
    """Sort/pad edges per core, pre-gather scaled source features.

    Returns per-core dicts {z, xt} plus the shared group structure g_run, G.
    """
    src = np.concatenate([edge_index[0], edge_index[1]]).astype(np.int64)
    dst = np.concatenate([edge_index[1], edge_index[0]]).astype(np.int64)
    rel = np.concatenate([edge_type, edge_type + NUM_REL]).astype(np.int64)

    core = dst // NPC
    dst_local = dst - core * NPC
    key = (dst_local // P) * R2 + rel          # run id within core

    run_counts = np.zeros((NCORES, NRUNS), dtype=np.int64)
    per_core = []
    for c in range(NCORES):
        m = core == c
        s_c, dl_c, k_c = src[m], dst_local[m], key[m]
        order = np.argsort(k_c, kind="stable")
        s_c, dl_c, k_c = s_c[order], dl_c[order], k_c[order]
        run_counts[c] = np.bincount(k_c, minlength=NRUNS)
        per_core.append((s_c, dl_c, k_c))

    maxcnt = np.max(run_counts, axis=0)                  # shared across cores
    g_run = (maxcnt + P - 1) // P
    # split each run into (g_run-1) full groups + one tail segment; pack the
    # 16 tails of every tile into shared "bin" groups (first-fit, run order).
    full_f = np.maximum(g_run - 1, 0).astype(np.int64)
    tail_sz = (maxcnt - full_f * P).astype(np.int64)     # in (0, P]
    full_g0 = np.zeros(NRUNS, np.int64)
    tail_g = np.zeros(NRUNS, np.int64)
    tail_off = np.zeros(NRUNS, np.int64)
    # one accumulation chain per PSUM partition-row region may be open at a
    # time (matmul start=True lazily marks the full 2KB bank row pending-zero)
    # so each run's groups stay contiguous: fulls then its own tail group.
    gidx = 0
    for kr in range(NRUNS):
        full_g0[kr] = gidx
        gidx += int(full_f[kr])
        if tail_sz[kr] > 0:
            tail_g[kr] = gidx
            gidx += 1
        else:
            tail_g[kr] = -1
    G = int(gidx)
    GP = ((G + J - 1) // J) * J
    layout = dict(full_f=full_f, full_g0=full_g0, tail_g=tail_g,
                  tail_off=tail_off, tail_sz=tail_sz, G=G, GP=GP)

    xf = x.astype(np.float32)
    cores_data = []
    for c in range(NCORES):
        s_c, dl_c, k_c = per_core[c]
        cnt_c = run_counts[c]
        run_starts = np.concatenate([[0], np.cumsum(cnt_c)])[:-1]
        rank = np.arange(len(k_c)) - run_starts[k_c]
        nf = full_f[k_c] * P
        slot = np.where(
            rank < nf,
            full_g0[k_c] * P + rank,
            tail_g[k_c] * P + tail_off[k_c] + (rank - nf))   # global slot id
        dst_in_tile = dl_c - (k_c // R2) * P             # 0..127
        # per-(rel,dst) counts -> fold 1/cnt into the gathered features
        subkey = k_c * P + dst_in_tile
        cnt_edge = np.bincount(subkey, minlength=NRUNS * P)[subkey]
        vals = xf[s_c] * (1.0 / cnt_edge)[:, None].astype(np.float32)

        # chunk layout: [J groups x 64 value cols | J dst f16 | J dst f32]
        zv = np.zeros((GP * P, IN_C), dtype=np.float16)
        zv[slot, :] = vals.astype(np.float16)
        zd = np.zeros((GP, P), dtype=np.float32)   # [group, slot] dst codes
        zd[slot // P, slot % P] = dst_in_tile.astype(np.float32)
        NCH = GP // J
        zv = zv.reshape(NCH, J, P, IN_C).transpose(0, 2, 1, 3).reshape(NCH, P, J * IN_C)
        zd16 = zd.astype(np.float16).reshape(NCH, J, P).transpose(0, 2, 1)
        z = np.ascontiguousarray(
            np.concatenate([zv, zd16], axis=2).transpose(1, 0, 2)
        ).reshape(P, GP * CW)

        xt = np.zeros((IN_C, NPAD), dtype=np.float16)
        xt[:, :NPC] = xf[c * NPC:(c + 1) * NPC].T
        cores_data.append({"z": z, "xt": xt})
    return cores_data, layout


def _build_program(layout, repeat=1, oh_pool_every=0, dbg_const_oh=False,
                   dbg_no_mm=False):
    import concourse.tile as tile
    from concourse import bass, bacc, mybir
    from contextlib import ExitStack

    f32 = mybir.dt.float32
    f16 = mybir.dt.float16
    GP = layout["GP"]
    nc = bacc.Bacc("TRN2", target_bir_lowering=False, debug=False,
                   num_devices=NCORES)

    z_dram = nc.declare_dram_parameter("z", [P, GP * CW], f16, isOutput=False)
    xt_dram = nc.declare_dram_parameter("xt", [IN_C, NPAD], f16, isOutput=False)
    wgt_dram = nc.declare_dram_parameter("wgt", [NUM_BASES, IN_C * OUT_C], f32, isOutput=False)
    mct_dram = nc.declare_dram_parameter("mct", [NUM_BASES, R2], f32, isOutput=False)
    root_dram = nc.declare_dram_parameter("root", [IN_C, OUT_C], f16, isOutput=False)
    bias_dram = nc.declare_dram_parameter("bias", [OUT_C, 1], f32, isOutput=False)
    iota_dram = nc.declare_dram_parameter("iota", [P, OB * P], f16, isOutput=False)  # repeated iota
    out_dram = nc.declare_dram_parameter("out", [OUT_C, NPAD], f32, isOutput=True)

    w_scratch = nc.dram_tensor("w_scratch", [R2, IN_C * OUT_C], f32)

    full_f, full_g0 = layout["full_f"], layout["full_g0"]
    tail_g, tail_off, tail_sz = layout["tail_g"], layout["tail_off"], layout["tail_sz"]
    NPAIR = R2 // 2                       # 8 pairs -> 2 quads of 4

    with tile.TileContext(nc) as tc:
        with ExitStack() as ctx:
            const_p = ctx.enter_context(tc.tile_pool(name="const", bufs=1, space="SBUF"))
            zchunk_p = ctx.enter_context(tc.tile_pool(name="zchunk", bufs=4, space="SBUF"))
            oh_p = ctx.enter_context(tc.tile_pool(name="oh", bufs=8, space="SBUF"))
            agg_p = ctx.enter_context(tc.tile_pool(name="agg", bufs=3, space="SBUF"))
            out_p = ctx.enter_context(tc.tile_pool(name="outs", bufs=2, space="SBUF"))
            ps_quad_p = ctx.enter_context(tc.tile_pool(name="psquad", bufs=4, space="PSUM"))
            ps_out_p = ctx.enter_context(tc.tile_pool(name="psout", bufs=2, space="PSUM"))

            iota_t = const_p.tile([P, OB * P], f16)
            nc.sync.dma_start(out=iota_t[:], in_=iota_dram[:])
            iota1_t = const_p.tile([P, P], f16)   # plain 0..127 rows, for Pool
            nc.vector.tensor_copy(out=iota1_t[:], in_=iota_t[:, ::OB])
            root_t = const_p.tile([IN_C, OUT_C], f16)
            nc.sync.dma_start(out=root_t[:], in_=root_dram[:])
            bias_t = const_p.tile([OUT_C, 1], f32)
            nc.sync.dma_start(out=bias_t[:], in_=bias_dram[:])

            # ---- weight composition: W[r] = ((mask*comp) @ weight_flat)[r] ----
            mct_t = const_p.tile([NUM_BASES, R2], f32)
            nc.sync.dma_start(out=mct_t[:], in_=mct_dram[:])
            wgt_t = const_p.tile([NUM_BASES, IN_C * OUT_C], f32)
            nc.sync.dma_start(out=wgt_t[:], in_=wgt_dram[:])
            w_all = const_p.tile([R2, IN_C * OUT_C], f32)
            for k in range(IN_C * OUT_C // 512):
                ps_w = ps_quad_p.tile([R2, 512], f32, space="PSUM", name="ps_quad")
                nc.tensor.matmul(out=ps_w[:], lhsT=mct_t[:],
                                 rhs=wgt_t[:, k * 512:(k + 1) * 512],
                                 start=True, stop=True)
                nc.vector.tensor_copy(out=w_all[:, k * 512:(k + 1) * 512], in_=ps_w[:])
            nc.sync.dma_start(out=w_scratch[:, :], in_=w_all[:])
            w_pairs = []
            for pp in range(NPAIR):
                wp32 = const_p.tile([2 * IN_C, OUT_C], f32, name=f"wp32_{pp}")
                nc.sync.dma_start(out=wp32[:], in_=w_scratch[2 * pp:2 * pp + 2, :])
                wp16 = const_p.tile([2 * IN_C, OUT_C], f16, name=f"wp16_{pp}")
                nc.vector.tensor_copy(out=wp16[:], in_=wp32[:])
                w_pairs.append(wp16)

            # whole transposed own-x slab stays resident (fp16, 25KB/partition)
            xt_t = const_p.tile([IN_C, NPAD], f16)
            nc.sync.dma_start(out=xt_t[:], in_=xt_dram[:])

            oh_const = None
            if dbg_const_oh:  # timing diagnostics only: skip one-hot builds
                oh_const = const_p.tile([P, OB * P], f16)
                nc.vector.tensor_copy(out=oh_const[:], in_=iota_t[:])

            # ---- main loop ----
            for rep in range(repeat):
                zt = None
                zt_ch = -1
                ohb = None
                o_sb = None
                nbld = 0
                def touch_group(g):
                    """Ensure chunk DMA + prefetched one-hot builds for the
                    whole chunk; return (zt, gl, oh_ap [P,P]-col view)."""
                    nonlocal zt, zt_ch, ohb, nbld
                    ch, gl = g // J, g % J
                    if ch != zt_ch:
                        zt = zchunk_p.tile([P, J * CW], f16, name="zt")
                        nc.sync.dma_start(
                            out=zt[:],
                            in_=z_dram[:, ch * J * CW:(ch + 1) * J * CW])
                        zt_ch = ch
                        if not dbg_const_oh:
                            # emit ALL builds of the chunk up front so DVE
                            # runs ahead of the consuming PE matmuls
                            ohb = []
                            for b in range(J // OB):
                                ob_t = oh_p.tile([P, OB * P], f16, name="ohb")
                                dstv = zt[:, J * IN_C + b * OB:
                                          J * IN_C + (b + 1) * OB]
                                nc.vector.tensor_tensor(
                                    out=ob_t[:], in0=iota_t[:],
                                    in1=dstv.unsqueeze(1)
                                            .broadcast_to([P, P, OB]),
                                    op=mybir.AluOpType.is_equal)
                                ohb.append(ob_t)
                                nbld += 1
                    if dbg_const_oh:
                        return zt, gl, oh_const[:, 0::OB]
                    return zt, gl, ohb[gl // OB][:, (gl % OB)::OB]

                for t in range(NTILES):
                    if t % 4 == 0:
                        o_sb = out_p.tile([OUT_C, 4 * P], f32, name="o_sb")
                    ps_out = ps_out_p.tile([OUT_C, P], f32, space="PSUM", name="ps_out")
                    has_r = [int(full_f[t * R2 + r]) > 0 or int(tail_sz[t * R2 + r]) > 0
                             for r in range(R2)]
                    n_mms = sum(1 for pair in range(NPAIR)
                                if has_r[2 * pair] or has_r[2 * pair + 1])
                    nc.tensor.matmul(out=ps_out[:], lhsT=root_t[:],
                                     rhs=xt_t[:, t * P:(t + 1) * P],
                                     start=True, stop=(n_mms == 0))
                    ps_quads = [ps_quad_p.tile([P, 4 * P], f32, space="PSUM",
                                               name="ps_quad") for _ in range(2)]

                    def region(r, ps_quads=ps_quads):
                        half, pr = r % 2, r // 2
                        return ps_quads[pr // 4][half * IN_C:(half + 1) * IN_C,
                                                 (pr % 4) * P:(pr % 4 + 1) * P]

                    # sequential chain per run: fulls then own tail group
                    for r in range(R2):
                        kr = t * R2 + r
                        for k in range(int(full_f[kr])):
                            ztl, gl, oh_ap = touch_group(int(full_g0[kr]) + k)
                            nc.tensor.matmul(
                                out=region(r),
                                lhsT=ztl[:, gl * IN_C:(gl + 1) * IN_C],
                                rhs=oh_ap,
                                start=(k == 0), stop=False)
                        s = int(tail_sz[kr])
                        if s == 0:
                            continue
                        a = int(tail_off[kr])
                        ztl, gl, oh_ap = touch_group(int(tail_g[kr]))
                        nc.tensor.matmul(
                            out=region(r),
                            lhsT=ztl[a:a + s, gl * IN_C:(gl + 1) * IN_C],
                            rhs=oh_ap[a:a + s, :],
                            start=(int(full_f[kr]) == 0), stop=True)
                    # tails done: copy quads, accumulate W matmuls
                    mm_i = 0
                    for q in range(2):
                        quad_pairs = [q * 4 + i for i in range(4)]
                        if not any(has_r[2 * pp] or has_r[2 * pp + 1]
                                   for pp in quad_pairs):
                            continue
                        agg_sb = agg_p.tile([P, 4 * P], f16, name="agg_sb")
                        nc.scalar.activation(
                            out=agg_sb[:], in_=ps_quads[q][:],
                            func=mybir.ActivationFunctionType.Copy)
                        for qi, pair in enumerate(quad_pairs):
                            h0, h1 = has_r[2 * pair], has_r[2 * pair + 1]
                            if not (h0 or h1):
                                continue
                            mm_i += 1
                            stop = (mm_i == n_mms)
                            if h0 and h1:
                                nc.tensor.matmul(
                                    out=ps_out[:], lhsT=w_pairs[pair],
                                    rhs=agg_sb[:, qi * P:(qi + 1) * P],
                                    start=False, stop=stop)
                            else:
                                half = 0 if h0 else 1
                                nc.tensor.matmul(
                                    out=ps_out[:],
                                    lhsT=w_pairs[pair][half * IN_C:(half + 1) * IN_C, :],
                                    rhs=agg_sb[half * IN_C:(half + 1) * IN_C,
                                               qi * P:(qi + 1) * P],
                                    start=False, stop=stop)
                    tq = t % 4
                    nc.scalar.activation(
                        out=o_sb[:, tq * P:(tq + 1) * P], in_=ps_out[:],
                        func=mybir.ActivationFunctionType.Identity,
                        bias=bias_t[:, 0:1])
                    if tq == 3 or t == NTILES - 1:
                        t0 = t - tq
                        nc.sync.dma_start(
                            out=out_dram[:, t0 * P:(t + 1) * P],
                            in_=o_sb[:, :(tq + 1) * P])

    nc.compile()
    return nc


def prepare(x, edge_index, edge_type, weight, comp, root, bias,
            repeat=1, oh_pool_every=0, dbg_const_oh=False, dbg_no_mm=False):
    x = np.asarray(x, dtype=np.float32)
    edge_index = np.asarray(edge_index)
    edge_type = np.asarray(edge_type)
    weight = np.asarray(weight, dtype=np.float32)
    comp = np.asarray(comp, dtype=np.float32)
    root = np.asarray(root, dtype=np.float32)
    bias = np.asarray(bias, dtype=np.float32)

    cores_data, layout = _host_prep(x, edge_index, edge_type)
    nc = _build_program(layout, repeat=repeat, oh_pool_every=oh_pool_every,
                        dbg_const_oh=dbg_const_oh, dbg_no_mm=dbg_no_mm)

    mask = _build_weight_mask()
    mct = np.ascontiguousarray((mask * comp).T)          # [36, 16]
    wgt = weight.reshape(NUM_BASES, IN_C * OUT_C)
    iota = np.tile(np.repeat(np.arange(P, dtype=np.float16), OB)[None, :], (P, 1))
    bias_col = bias.reshape(OUT_C, 1)

    in_maps = []
    for c in range(NCORES):
        d = cores_data[c]
        in_maps.append({
            "z": d["z"], "xt": d["xt"], "wgt": wgt, "mct": mct,
            "root": root.astype(np.float16), "bias": bias_col, "iota": iota,
        })
    return nc, in_maps


def assemble(results):
    out = np.empty((N_NODES, OUT_C), dtype=np.float32)
    for c in range(NCORES):
        out[c * NPC:(c + 1) * NPC] = results[c]["out"][:, :NPC].T
    return out


def kernel(x, edge_index, edge_type, weight, comp, root, bias):
    from concourse.bass_utils import run_bass_kernel_spmd

    nc, in_maps = prepare(x, edge_index, edge_type, weight, comp, root, bias)
    res = run_bass_kernel_spmd(nc, in_maps, core_ids=list(range(NCORES)))
    return assemble(res.results)
